# revision 8
# baseline (speedup 1.0000x reference)
"""Trainium2 Bass kernel for nn_BinaryLabelSoftRouter.

Reference computation (B=16, T=2048, D=2048, H=256):
    base = lookup[labels]                                   (B,T,2)
    h = gelu(LN(x @ W1 + b1) * g1 + bt1)
    h = gelu(LN(h @ W2 + b2) * g2 + bt2)
    adj = tanh(h @ W3 + b3) * 0.1
    adjusted = softmax((base + adj) / clip(temp, 0.1))      (B,T,2)
    final = EMA scan over T (s_t = 0.9 s_{t-1} + 0.1 c_t)   (B,T,2)
    returns (final, base, adjusted)

Strategy: data-parallel over B across 8 NeuronCores (2 batches/core).
v2 design (vs the 113.7us baseline):
  - x is quantized to fp8e4m3 and pre-transposed on the host to
    [128, KC, TOK]; layer-1 runs fp8 DoubleRow matmuls (K=256 per
    instruction at 0.5 cycles/row) -> ~4x less PE time and ~2x less
    DMA-device time than bf16 + on-device XBAR transpose.
    W1 is scaled by 128 before the fp8 cast to lift it out of the fp8
    subnormal range; layernorm makes the result scale-invariant (b1 is
    scaled identically when nonzero).
  - h1/h2 pre-activations stay resident in PSUM; the scalar engine
    applies LN+GELU directly from PSUM (no DVE stash copies).
  - inverse LN sigma comes from one DVE tensor_scalar (v+eps)^-0.5
    (AluOpType.pow), so the ACT engine only ever needs the
    gelu_and_others table (Gelu+Tanh+Identity): one table load total.
  - the per-tile softmax tail is batched per 4-tile quarter: one Tanh
    over [128,8], strided DVE ops, one Tanh over [128,8].
  - the EMA block-scan carry is computed entirely with tiny matmuls
    (carry row u = A^T ccat, then an in-row convolution with powers of
    0.9^128) -- no DRAM round-trip.
"""

import sys

sys.path.insert(0, "/opt/trn_rl_repo")

import numpy as np
import ml_dtypes

import concourse.bass as bass
import concourse.mybir as mybir
from concourse import bacc
from concourse.bass import ts
from concourse.tile import TileContext
from concourse.bass_utils import run_bass_kernel_spmd

F32 = mybir.dt.float32
U32 = mybir.dt.uint32
BF16 = mybir.dt.bfloat16
FP8 = mybir.dt.float8e4
AFT = mybir.ActivationFunctionType
ALU = mybir.AluOpType
PM = mybir.MatmulPerfMode
BF = ml_dtypes.bfloat16
F8 = ml_dtypes.float8_e4m3

B, T, D, H = 16, 2048, 2048, 256
H2 = H // 2
ADJ = 0.1
SMOOTH = 0.9
EPS = 1e-5
N_CORES = 8
BPC = B // N_CORES           # batches per core
TOK = BPC * T                # tokens per core
NT = TOK // 128              # 128-token tiles per core (32)
NTB = T // 128               # tiles per batch (16)
KC = D // 128                # k-chunks of 128 for layer 1 (16)
QT = 4                       # tiles per quarter
NQ = NT // QT                # quarters (8)
TG = QT * 128                # tokens per x slab (512)
W1SCALE = 128.0              # fp8 range lift for W1 (LN removes it)
EPS1 = EPS * W1SCALE * W1SCALE   # ln eps in the scaled-h1 domain


def _build_nc(flags):
    nz_b1 = flags["nz_b1"]
    nz_b2 = flags["nz_b2"]
    nz_b3 = flags["nz_b3"]
    gb1 = flags["gb1"]
    gb2 = flags["gb2"]
    sig_scale = flags["sig_scale"]   # 0.1 / temp

    nc = bacc.Bacc("TRN2", target_bir_lowering=False)

    x_d = nc.dram_tensor("x", [128, KC, TOK], FP8, kind="ExternalInput")
    w1_d = nc.dram_tensor("w1", [128, KC, H], FP8, kind="ExternalInput")
    w2_d = nc.dram_tensor("w2", [128, 2, H2], BF16, kind="ExternalInput")
    w3_d = nc.dram_tensor("w3", [128, 2], BF16, kind="ExternalInput")
    labt_d = nc.dram_tensor("labt", [128, NT], F32, kind="ExternalInput")
    ladj_d = nc.dram_tensor("ladj", [128, NT], F32, kind="ExternalInput")
    nladj_d = nc.dram_tensor("nladj", [128, NT], F32, kind="ExternalInput")
    prev_d = nc.dram_tensor("prevr", [1, 2 * BPC], F32, kind="ExternalInput")
    t0t_d = nc.dram_tensor("t0t", [128, 128], F32, kind="ExternalInput")
    pvec_d = nc.dram_tensor("pvec", [1, 128], F32, kind="ExternalInput")
    avec_d = nc.dram_tensor("avec", [128, 1], F32, kind="ExternalInput")
    ident_d = nc.dram_tensor("ident", [128, 128], BF16, kind="ExternalInput")
    if nz_b1 or nz_b2 or nz_b3:
        ones_d = nc.dram_tensor("onesr", [1, 128], BF16, kind="ExternalInput")
    if nz_b1:
        b1_d = nc.dram_tensor("b1r", [1, H], BF16, kind="ExternalInput")
    if nz_b2:
        b2_d = nc.dram_tensor("b2r", [1, H2], BF16, kind="ExternalInput")
    if nz_b3:
        b3_d = nc.dram_tensor("b3r", [1, 2], BF16, kind="ExternalInput")
    if gb1:
        g1_d = nc.dram_tensor("g1f", [128, H], F32, kind="ExternalInput")
        bt1_d = nc.dram_tensor("bt1f", [128, H], F32, kind="ExternalInput")
    if gb2:
        g2_d = nc.dram_tensor("g2f", [128, H2], F32, kind="ExternalInput")
        bt2_d = nc.dram_tensor("bt2f", [128, H2], F32, kind="ExternalInput")

    fin_d = nc.dram_tensor("fin", [128, 2 * NT], F32, kind="ExternalOutput")
    bas_d = nc.dram_tensor("bas", [128, 2 * NT], F32, kind="ExternalOutput")
    adw_d = nc.dram_tensor("adw", [128, 2 * NT], F32, kind="ExternalOutput")

    with TileContext(nc) as tc:
        with (
            tc.tile_pool(name="consts", bufs=1) as cpool,
            tc.tile_pool(name="xt", bufs=8) as xtpool,
            tc.tile_pool(name="stash", bufs=1) as hpool,
            tc.tile_pool(name="work", bufs=4) as wpool,
            tc.tile_pool(name="small", bufs=8) as spool,
            tc.tile_pool(name="ph1", bufs=2, space="PSUM") as ph1pool,
            tc.tile_pool(name="ph2", bufs=2, space="PSUM") as ph2pool,
            tc.tile_pool(name="pl3", bufs=1, space="PSUM") as pl3pool,
            tc.tile_pool(name="pema", bufs=1, space="PSUM") as pemapool,
        ):
            # ---- constants into SBUF on the sync HWDGE queue; the tile
            # scheduler interleaves them with the x slabs, and keeping them
            # off the Activation queue frees its first ~6us for GELUs
            def cload(shape, dt, dram, tag):
                t = cpool.tile(shape, dt, tag=tag)
                sl = tuple(slice(None) for _ in shape)
                nc.sync.dma_start(t[sl], dram[sl])
                return t

            w1s = cload([128, KC, H], FP8, w1_d, tag="w1s")
            w2s = cload([128, 2, H2], BF16, w2_d, tag="w2s")
            w3s = cload([128, 2], BF16, w3_d, tag="w3s")
            labts = cload([128, NT], F32, labt_d, tag="labts")
            ladjs = cload([128, NT], F32, ladj_d, tag="ladjs")
            nladjs = cload([128, NT], F32, nladj_d, tag="nladjs")
            prevs = cload([1, 2 * BPC], F32, prev_d, tag="prevs")
            t0ts = cload([128, 128], F32, t0t_d, tag="t0ts")
            pvecs = cload([1, 128], F32, pvec_d, tag="pvecs")
            avecs = cload([128, 1], F32, avec_d, tag="avecs")
            ident128 = cload([128, 128], BF16, ident_d, tag="ident128")
            oness = cload([1, 128], BF16, ones_d, tag="oness") if (nz_b1 or nz_b2 or nz_b3) else None
            b1s = cload([1, H], BF16, b1_d, tag="b1s") if nz_b1 else None
            b2s = cload([1, H2], BF16, b2_d, tag="b2s") if nz_b2 else None
            b3s = cload([1, 2], BF16, b3_d, tag="b3s") if nz_b3 else None
            g1s = cload([128, H], F32, g1_d, tag="g1s") if gb1 else None
            bt1s = cload([128, H], F32, bt1_d, tag="bt1s") if gb1 else None
            g2s = cload([128, H2], F32, g2_d, tag="g2s") if gb2 else None
            bt2s = cload([128, H2], F32, bt2_d, tag="bt2s") if gb2 else None

            # preload the one activation table that covers every function
            # this kernel uses (Gelu, Tanh, Identity) so the table-load pass
            # doesn't bounce through a Tanh-only table first
            nc.scalar.add_instruction(
                mybir.InstLoadActFuncSet(
                    name=nc.get_next_instruction_name(),
                    act_func_set_id=10,   # gelu_and_others
                    ins=[], outs=[],
                )
            )

            # ---- long-lived buffers
            rsqk = cpool.tile([128, QT], U32, tag="rsqk")
            nc.vector.memset(rsqk[:, :], 0x5F3759DF)
            ccat = hpool.tile([128, NT, 2], F32, tag="ccat")
            bases = hpool.tile([128, NT, 2], F32, tag="bases")
            finals = hpool.tile([128, NT, 2], F32, tag="finals")
            adjts = hpool.tile([128, NT, 2], F32, tag="adjts")
            zall = hpool.tile([128, NT, 2], F32, tag="zall")
            diffs = hpool.tile([128, NT], F32, tag="diffs")
            istd1 = hpool.tile([128, NT], F32, tag="istd1")
            nms1 = hpool.tile([128, NT], F32, tag="nms1")
            istd2 = hpool.tile([128, NT], F32, tag="istd2")
            nms2 = hpool.tile([128, NT], F32, tag="nms2")

            def sigma(st, istd_out, nms_out, n, eps, tag):
                """LN istd and -mean*istd for one quarter from bn_stats'
                per-tile (even,odd) half-stats.  Welford merge:
                M2 = M2e + M2o + (n/4)*(me-mo)^2 ; var = M2/n ;
                mean = (me+mo)/2.  istd = (var+eps)^-0.5 via magic-constant
                guess + one Newton step (rel err <= ~0.2%, far below the
                fp8 noise floor -- no ACT Sqrt table, one DVE op only)."""
                d = spool.tile([128, QT], F32, tag=tag + "d")
                nc.gpsimd.tensor_sub(d[:, :], st[:, :, 1], st[:, :, 4])
                nc.gpsimd.tensor_mul(d[:, :], d[:, :], d[:, :])
                nc.gpsimd.tensor_scalar_mul(d[:, :], d[:, :], n / 4.0)
                nc.gpsimd.tensor_add(d[:, :], d[:, :], st[:, :, 2])
                nc.gpsimd.tensor_add(d[:, :], d[:, :], st[:, :, 5])
                vp = spool.tile([128, QT], F32, tag=tag + "vp")
                nc.gpsimd.tensor_scalar(vp[:, :], d[:, :], 1.0 / n, eps, ALU.mult, ALU.add)
                y1 = spool.tile([128, QT], U32, tag=tag + "y1")
                nc.vector.tensor_scalar(
                    y1[:, :], vp[:, :].bitcast(U32), 1, None,
                    ALU.arith_shift_right,
                )
                y0u = spool.tile([128, QT], U32, tag=tag + "y0")
                nc.gpsimd.tensor_sub(y0u[:, :], rsqk[:, :], y1[:, :])
                y0 = y0u[:, :].bitcast(F32)
                t1 = spool.tile([128, QT], F32, tag=tag + "t1")
                nc.gpsimd.tensor_mul(t1[:, :], y0, y0)
                nc.gpsimd.tensor_mul(t1[:, :], t1[:, :], vp[:, :])
                nc.gpsimd.tensor_scalar(t1[:, :], t1[:, :], -0.5, 1.5, ALU.mult, ALU.add)
                nc.gpsimd.tensor_mul(istd_out, y0, t1[:, :])
                m = spool.tile([128, QT], F32, tag=tag + "m")
                nc.gpsimd.tensor_add(m[:, :], st[:, :, 1], st[:, :, 4])
                nc.gpsimd.tensor_mul(m[:, :], m[:, :], istd_out)
                nc.gpsimd.tensor_scalar_mul(nms_out, m[:, :], -0.5)

            def ema_batch(b):
                """Block-scan + final smoothing for batch b, all on PE.
                u[2j+ch] = sum_p A[p]*ccat[p, 16b+j, ch] is block j's
                end-of-block EMA contribution; the carry entering block j is
                S_{j-1} = u_{j-1} + 0.9^128 * S_{j-2}.  0.9^128 = 1.4e-6 and
                |S| <= 1, so the recursive term is below the fp8 noise floor
                by 3 orders of magnitude: the carry is just u shifted by one
                block (with `prev` entering block 0).
                finals = T0^T @ ccat + pvec (x) carry."""
                cslice = ccat[:, 16 * b : 16 * (b + 1), :]
                ema = pemapool.tile([128, 2, NTB, 2], F32, tag="ema")
                u_p = ema[:, 0]
                pfin = ema[:, 1]
                nc.tensor.matmul(
                    u_p[0:1, :, :], avecs[:, :], cslice,
                    start=True, stop=True,
                )
                u_sb = spool.tile([1, NTB, 2], F32, tag="u_sb")
                nc.vector.tensor_copy(u_sb[0:1, :, :], u_p[0:1, :, :])

                nc.tensor.matmul(
                    pfin[:, :, :], t0ts[:, :], cslice,
                    start=True, stop=False,
                )
                nc.tensor.matmul(
                    pfin[:, 1:NTB, :], pvecs[0:1, :], u_sb[0:1, 0 : NTB - 1, :],
                    start=False, stop=False,
                )
                nc.tensor.matmul(
                    pfin[:, 0:1, :], pvecs[0:1, :],
                    prevs[0:1, 2 * b : 2 * (b + 1)],
                    start=False, stop=True,
                )
                nc.vector.tensor_copy(finals[:, 16 * b : 16 * (b + 1), :], pfin[:, :, :])

            for q in range(NQ):
                q4 = q * QT
                xt = xtpool.tile([128, KC, TG], FP8, tag="xt")
                nc.sync.dma_start(xt[:, :, :], x_d[:, :, q * TG : (q + 1) * TG])

                # ---- layer 1 (fp8 DoubleRow) + stats
                ph1 = ph1pool.tile([128, QT, H], F32)
                st1 = spool.tile([128, QT, 6], F32, tag="bnst")
                for j in range(QT):
                    for kk in range(KC // 2):
                        nc.tensor.matmul(
                            ph1[:, j, :],
                            xt[:, 2 * kk : 2 * kk + 2, ts(j, 128)],
                            w1s[:, 2 * kk : 2 * kk + 2, :],
                            start=(kk == 0),
                            stop=(kk == KC // 2 - 1 and not nz_b1),
                            perf_mode=PM.DoubleRow,
                        )
                    if nz_b1:
                        nc.tensor.matmul(
                            ph1[:, j, :], oness[:, :], b1s[:, :],
                            start=False, stop=True,
                        )
                    nc.vector.bn_stats(st1[:, j, :], ph1[:, j, :])

                # ---- LN1 sigma for the quarter (gpsimd Welford merge of
                # bn_stats' even/odd halves; no ACT table, almost no DVE)
                sl = slice(q4, q4 + QT)
                sigma(st1, istd1[:, sl], nms1[:, sl], H, EPS1, "s1")

                # ---- LN1-apply + GELU (ACT from PSUM), transpose, layer 2
                ph2 = ph2pool.tile([128, QT, H2], F32)
                st2 = spool.tile([128, QT, 6], F32, tag="bnst2")
                for j in range(QT):
                    i = q4 + j
                    h1g = wpool.tile([128, H], BF16, tag="h1g")
                    if not gb1:
                        nc.scalar.activation(
                            h1g[:, :], ph1[:, j, :], AFT.Gelu,
                            bias=nms1[:, i : i + 1], scale=istd1[:, i : i + 1],
                        )
                    else:
                        tmp = spool.tile([128, H], F32, tag="lng1")
                        nc.scalar.activation(
                            tmp[:, :], ph1[:, j, :], AFT.Identity,
                            bias=nms1[:, i : i + 1], scale=istd1[:, i : i + 1],
                        )
                        nc.vector.tensor_mul(tmp[:, :], tmp[:, :], g1s[:, :])
                        nc.vector.tensor_add(tmp[:, :], tmp[:, :], bt1s[:, :])
                        nc.scalar.activation(h1g[:, :], tmp[:, :], AFT.Gelu)
                    h1gt = wpool.tile([128, 2, H2], BF16, tag="h1gt")
                    nc.sync.dma_start(h1gt[:, :, :], h1g[:, :], transpose=True)
                    for hh in range(2):
                        nc.tensor.matmul(
                            ph2[:, j, :], h1gt[:, hh, :], w2s[:, hh, :],
                            start=(hh == 0), stop=(hh == 1 and not nz_b2),
                        )
                    if nz_b2:
                        nc.tensor.matmul(
                            ph2[:, j, :], oness[:, :], b2s[:, :],
                            start=False, stop=True,
                        )
                    nc.vector.bn_stats(st2[:, j, :], ph2[:, j, :])

                # ---- LN2 sigma
                sigma(st2, istd2[:, sl], nms2[:, sl], H2, EPS, "s2")

                # ---- LN2-apply + GELU, transpose (PE), layer 3
                pl3t = pl3pool.tile([128, QT, 72], F32)
                pl3 = pl3t[:, :, 0:2]
                for j in range(QT):
                    i = q4 + j
                    h2g = wpool.tile([128, H2], BF16, tag="h2g")
                    if not gb2:
                        nc.scalar.activation(
                            h2g[:, :], ph2[:, j, :], AFT.Gelu,
                            bias=nms2[:, i : i + 1], scale=istd2[:, i : i + 1],
                        )
                    else:
                        tmp = spool.tile([128, H2], F32, tag="lng2")
                        nc.scalar.activation(
                            tmp[:, :], ph2[:, j, :], AFT.Identity,
                            bias=nms2[:, i : i + 1], scale=istd2[:, i : i + 1],
                        )
                        nc.vector.tensor_mul(tmp[:, :], tmp[:, :], g2s[:, :])
                        nc.vector.tensor_add(tmp[:, :], tmp[:, :], bt2s[:, :])
                        nc.scalar.activation(h2g[:, :], tmp[:, :], AFT.Gelu)
                    h2gtp = pl3t[:, j, 8:72].bitcast(BF16)
                    nc.tensor.matmul(
                        h2gtp, h2g[:, :], ident128[:, :],
                        is_transpose=True, start=True, stop=True,
                    )
                    h2gt = wpool.tile([128, H2], BF16, tag="h2gt")
                    nc.vector.tensor_copy(h2gt[:, :], h2gtp)
                    nc.tensor.matmul(
                        pl3[:, j, :], h2gt[:, :], w3s[:, :],
                        start=True, stop=not nz_b3,
                    )
                    if nz_b3:
                        nc.tensor.matmul(
                            pl3[:, j, :], oness[:, :], b3s[:, :],
                            start=False, stop=True,
                        )

                # ---- batched softmax tail for the quarter
                nc.scalar.activation(adjts[:, sl, :], pl3[:, :, :], AFT.Tanh)
                nc.gpsimd.tensor_sub(
                    diffs[:, sl], adjts[:, sl, 1], adjts[:, sl, 0]
                )
                nc.gpsimd.tensor_scalar(
                    zall[:, sl, 1], diffs[:, sl], 0.5 * sig_scale, None, ALU.mult
                )
                nc.gpsimd.tensor_add(zall[:, sl, 1], zall[:, sl, 1], ladjs[:, sl])
                nc.gpsimd.tensor_scalar(
                    zall[:, sl, 0], diffs[:, sl], -0.5 * sig_scale, None, ALU.mult
                )
                nc.gpsimd.tensor_add(zall[:, sl, 0], zall[:, sl, 0], nladjs[:, sl])
                th = spool.tile([128, QT, 2], F32, tag="th")
                nc.scalar.activation(th[:, :, :], zall[:, sl, :], AFT.Tanh)
                nc.gpsimd.tensor_scalar(
                    ccat[:, sl, :], th[:, :, :], 0.5, 0.5, ALU.mult, ALU.add
                )

                if (q + 1) % (NTB // QT) == 0:
                    ema_batch((q + 1) // (NTB // QT) - 1)

            # ---- base weights, batched over all tiles
            nc.gpsimd.tensor_scalar(
                bases[:, :, 0], labts[:, :], -0.5, 0.75, ALU.mult, ALU.add
            )
            nc.gpsimd.tensor_scalar(
                bases[:, :, 1], labts[:, :], 0.5, 0.25, ALU.mult, ALU.add
            )

            # ---- store outputs
            nc.sync.dma_start(fin_d[:, :], finals[:, :, :])
            nc.sync.dma_start(bas_d[:, :], bases[:, :, :])
            nc.scalar.dma_start(adw_d[:, :], ccat[:, :, :])

    nc.compile()
    return nc


_NC_CACHE = {}


def _get_nc(flags):
    key = tuple(sorted(flags.items()))
    if key not in _NC_CACHE:
        _NC_CACHE[key] = _build_nc(flags)
    return _NC_CACHE[key]


def _ema_constants():
    """Constant tensors for the matmul-based EMA block scan (fp32)."""
    s, o = SMOOTH, 1.0 - SMOOTH
    dt = np.arange(128)
    dk = np.arange(128)
    expo = dt[None, :] - dk[:, None]
    t0t = np.where(expo >= 0, o * np.power(s, np.clip(expo, 0, None)), 0.0)
    pvec = np.power(s, dt + 1.0)
    avec = o * np.power(s, 127.0 - dk)
    return (
        t0t.astype(np.float32),
        pvec.astype(np.float32).reshape(1, 128),
        avec.astype(np.float32).reshape(128, 1),
    )


def prepare(critical_labels, action_tokens, prev_weights,
            W1, b1, g1, bt1, W2, b2, g2, bt2, W3, b3, temperature):
    """Host-side marshalling. Returns (nc, in_maps, postprocess)."""
    labels = np.asarray(critical_labels)
    x = np.asarray(action_tokens, dtype=np.float32)
    prev = np.asarray(prev_weights, dtype=np.float32)
    W1 = np.asarray(W1, dtype=np.float32)
    W2 = np.asarray(W2, dtype=np.float32)
    W3 = np.asarray(W3, dtype=np.float32)
    b1 = np.asarray(b1, dtype=np.float32)
    b2 = np.asarray(b2, dtype=np.float32)
    b3 = np.asarray(b3, dtype=np.float32)
    g1 = np.asarray(g1, dtype=np.float32)
    bt1 = np.asarray(bt1, dtype=np.float32)
    g2 = np.asarray(g2, dtype=np.float32)
    bt2 = np.asarray(bt2, dtype=np.float32)
    temp = float(np.clip(np.asarray(temperature, dtype=np.float32), 0.1, None))
    inv_t = 1.0 / temp

    flags = {
        "nz_b1": bool(np.any(b1 != 0)),
        "nz_b2": bool(np.any(b2 != 0)),
        "nz_b3": bool(np.any(b3 != 0)),
        "gb1": bool(np.any(g1 != 1) or np.any(bt1 != 0)),
        "gb2": bool(np.any(g2 != 1) or np.any(bt2 != 0)),
        "sig_scale": float(ADJ * inv_t),
    }
    nc = _get_nc(flags)

    w1r = np.ascontiguousarray(
        (W1 * W1SCALE).astype(F8).reshape(KC, 128, H).transpose(1, 0, 2)
    )
    w2r = np.ascontiguousarray(
        W2.astype(BF).reshape(2, 128, H2).transpose(1, 0, 2)
    )
    w3r = np.ascontiguousarray(W3.astype(BF))
    t0t, pvec, avec = _ema_constants()
    shared = {
        "w1": w1r, "w2": w2r, "w3": w3r,
        "t0t": t0t, "pvec": pvec, "avec": avec,
        "ident": np.eye(128, dtype=BF),
    }
    if flags["nz_b1"] or flags["nz_b2"] or flags["nz_b3"]:
        shared["onesr"] = np.ones((1, 128), dtype=BF)
    if flags["nz_b1"]:
        shared["b1r"] = (b1 * W1SCALE).astype(BF).reshape(1, H)
    if flags["nz_b2"]:
        shared["b2r"] = b2.astype(BF).reshape(1, H2)
    if flags["nz_b3"]:
        shared["b3r"] = b3.astype(BF).reshape(1, 2)
    if flags["gb1"]:
        shared["g1f"] = np.broadcast_to(g1.reshape(1, H), (128, H)).copy()
        shared["bt1f"] = np.broadcast_to(bt1.reshape(1, H), (128, H)).copy()
    if flags["gb2"]:
        shared["g2f"] = np.broadcast_to(g2.reshape(1, H2), (128, H2)).copy()
        shared["bt2f"] = np.broadcast_to(bt2.reshape(1, H2), (128, H2)).copy()

    lab_f = labels.astype(np.float32).reshape(N_CORES, BPC * T)
    # fp8-quantize then transpose to [core, 128, KC, TOK]
    x8 = x.reshape(N_CORES, TOK, KC, 128).astype(F8)
    xt8 = np.ascontiguousarray(x8.transpose(0, 3, 2, 1))
    prev_r = prev.reshape(N_CORES, BPC * 2)

    in_maps = []
    for c in range(N_CORES):
        m = dict(shared)
        m["x"] = xt8[c]
        labt = np.ascontiguousarray(lab_f[c].reshape(NT, 128).T)
        m["labt"] = labt
        ladj = np.ascontiguousarray((labt - 0.5) * inv_t * 0.5)
        m["ladj"] = ladj
        m["nladj"] = np.ascontiguousarray(-ladj)
        m["prevr"] = prev_r[c : c + 1]
        in_maps.append(m)

    def postprocess(results):
        outs = []
        for name in ("fin", "bas", "adw"):
            per_core = []
            for c in range(N_CORES):
                a = results[c][name].reshape(128, NT, 2)
                per_core.append(
                    np.ascontiguousarray(a.transpose(1, 0, 2)).reshape(BPC, T, 2)
                )
            outs.append(np.concatenate(per_core, axis=0))
        return tuple(outs)   # (final, base, adjusted)

    return nc, in_maps, postprocess


def kernel(**inputs):
    nc, in_maps, postprocess = prepare(**inputs)
    res = run_bass_kernel_spmd(nc, in_maps, core_ids=list(range(N_CORES)))
    return postprocess(res.results)


# revision 9
# speedup vs baseline: 1.1221x; 1.1221x over previous
"""Trainium2 Bass kernel for nn_BinaryLabelSoftRouter.

Reference computation (B=16, T=2048, D=2048, H=256):
    base = lookup[labels]                                   (B,T,2)
    h = gelu(LN(x @ W1 + b1) * g1 + bt1)
    h = gelu(LN(h @ W2 + b2) * g2 + bt2)
    adj = tanh(h @ W3 + b3) * 0.1
    adjusted = softmax((base + adj) / clip(temp, 0.1))      (B,T,2)
    final = EMA scan over T (s_t = 0.9 s_{t-1} + 0.1 c_t)   (B,T,2)
    returns (final, base, adjusted)

Strategy: data-parallel over B across 8 NeuronCores (2 batches/core).
v2 design (vs the 113.7us baseline):
  - x is quantized to fp8e4m3 and pre-transposed on the host to
    [128, KC, TOK]; layer-1 runs fp8 DoubleRow matmuls (K=256 per
    instruction at 0.5 cycles/row) -> ~4x less PE time and ~2x less
    DMA-device time than bf16 + on-device XBAR transpose.
    W1 is scaled by 128 before the fp8 cast to lift it out of the fp8
    subnormal range; layernorm makes the result scale-invariant (b1 is
    scaled identically when nonzero).
  - h1/h2 pre-activations stay resident in PSUM; the scalar engine
    applies LN+GELU directly from PSUM (no DVE stash copies).
  - inverse LN sigma comes from one DVE tensor_scalar (v+eps)^-0.5
    (AluOpType.pow), so the ACT engine only ever needs the
    gelu_and_others table (Gelu+Tanh+Identity): one table load total.
  - the per-tile softmax tail is batched per 4-tile quarter: one Tanh
    over [128,8], strided DVE ops, one Tanh over [128,8].
  - the EMA block-scan carry is computed entirely with tiny matmuls
    (carry row u = A^T ccat, then an in-row convolution with powers of
    0.9^128) -- no DRAM round-trip.
"""

import sys

sys.path.insert(0, "/opt/trn_rl_repo")

import numpy as np
import ml_dtypes

import concourse.bass as bass
import concourse.mybir as mybir
from concourse import bacc
from concourse.bass import ts
from concourse.tile import TileContext
from concourse.bass_utils import run_bass_kernel_spmd

F32 = mybir.dt.float32
U32 = mybir.dt.uint32
BF16 = mybir.dt.bfloat16
FP8 = mybir.dt.float8e4
AFT = mybir.ActivationFunctionType
ALU = mybir.AluOpType
PM = mybir.MatmulPerfMode
BF = ml_dtypes.bfloat16
F8 = ml_dtypes.float8_e4m3

B, T, D, H = 16, 2048, 2048, 256
H2 = H // 2
ADJ = 0.1
SMOOTH = 0.9
EPS = 1e-5
N_CORES = 8
BPC = B // N_CORES           # batches per core
TOK = BPC * T                # tokens per core
NT = TOK // 128              # 128-token tiles per core (32)
NTB = T // 128               # tiles per batch (16)
KC = D // 128                # k-chunks of 128 for layer 1 (16)
QT = 4                       # tiles per quarter
NQ = NT // QT                # quarters (8)
TG = QT * 128                # tokens per x slab (512)
W1SCALE = 128.0              # fp8 range lift for W1 (LN removes it)
EPS1 = EPS * W1SCALE * W1SCALE   # ln eps in the scaled-h1 domain


def _build_nc(flags):
    nz_b1 = flags["nz_b1"]
    nz_b2 = flags["nz_b2"]
    nz_b3 = flags["nz_b3"]
    gb1 = flags["gb1"]
    gb2 = flags["gb2"]
    sig_scale = flags["sig_scale"]   # 0.1 / temp

    nc = bacc.Bacc("TRN2", target_bir_lowering=False)

    x_d = nc.dram_tensor("x", [128, KC, TOK], FP8, kind="ExternalInput")
    w1_d = nc.dram_tensor("w1", [128, KC, H], FP8, kind="ExternalInput")
    w2_d = nc.dram_tensor("w2", [128, 2, H2], BF16, kind="ExternalInput")
    w3_d = nc.dram_tensor("w3", [128, 2], BF16, kind="ExternalInput")
    labt_d = nc.dram_tensor("labt", [128, NT], F32, kind="ExternalInput")
    ladj_d = nc.dram_tensor("ladj", [128, NT], F32, kind="ExternalInput")
    nladj_d = nc.dram_tensor("nladj", [128, NT], F32, kind="ExternalInput")
    prev_d = nc.dram_tensor("prevr", [1, 2 * BPC], F32, kind="ExternalInput")
    t0t_d = nc.dram_tensor("t0t", [128, 128], F32, kind="ExternalInput")
    pvec_d = nc.dram_tensor("pvec", [1, 128], F32, kind="ExternalInput")
    avec_d = nc.dram_tensor("avec", [128, 1], F32, kind="ExternalInput")
    ident_d = nc.dram_tensor("ident", [128, 128], BF16, kind="ExternalInput")
    if nz_b1 or nz_b2 or nz_b3:
        ones_d = nc.dram_tensor("onesr", [1, 128], BF16, kind="ExternalInput")
    if nz_b1:
        b1_d = nc.dram_tensor("b1r", [1, H], BF16, kind="ExternalInput")
    if nz_b2:
        b2_d = nc.dram_tensor("b2r", [1, H2], BF16, kind="ExternalInput")
    if nz_b3:
        b3_d = nc.dram_tensor("b3r", [1, 2], BF16, kind="ExternalInput")
    if gb1:
        g1_d = nc.dram_tensor("g1f", [128, H], F32, kind="ExternalInput")
        bt1_d = nc.dram_tensor("bt1f", [128, H], F32, kind="ExternalInput")
    if gb2:
        g2_d = nc.dram_tensor("g2f", [128, H2], F32, kind="ExternalInput")
        bt2_d = nc.dram_tensor("bt2f", [128, H2], F32, kind="ExternalInput")

    fin_d = nc.dram_tensor("fin", [128, 2 * NT], F32, kind="ExternalOutput")
    bas_d = nc.dram_tensor("bas", [128, 2 * NT], F32, kind="ExternalOutput")
    adw_d = nc.dram_tensor("adw", [128, 2 * NT], F32, kind="ExternalOutput")

    with TileContext(nc) as tc:
        with (
            tc.tile_pool(name="consts", bufs=1) as cpool,
            tc.tile_pool(name="xt", bufs=8) as xtpool,
            tc.tile_pool(name="stash", bufs=1) as hpool,
            tc.tile_pool(name="work", bufs=4) as wpool,
            tc.tile_pool(name="small", bufs=8) as spool,
            tc.tile_pool(name="ph1", bufs=2, space="PSUM") as ph1pool,
            tc.tile_pool(name="ph2", bufs=2, space="PSUM") as ph2pool,
            tc.tile_pool(name="pl3", bufs=1, space="PSUM") as pl3pool,
            tc.tile_pool(name="pema", bufs=1, space="PSUM") as pemapool,
        ):
            # ---- constants into SBUF on the SWDGE queue: the Pool engine
            # is otherwise idle at kernel start, so its software descriptor
            # generation is free, and neither the sync queue (x slabs) nor
            # the Activation queue (GELUs) is delayed
            def cload(shape, dt, dram, tag):
                t = cpool.tile(shape, dt, tag=tag)
                sl = tuple(slice(None) for _ in shape)
                nc.gpsimd.dma_start(t[sl], dram[sl])
                return t

            w1s = cload([128, KC, H], FP8, w1_d, tag="w1s")
            w2s = cload([128, 2, H2], BF16, w2_d, tag="w2s")
            w3s = cload([128, 2], BF16, w3_d, tag="w3s")
            labts = cload([128, NT], F32, labt_d, tag="labts")
            ladjs = cload([128, NT], F32, ladj_d, tag="ladjs")
            nladjs = cload([128, NT], F32, nladj_d, tag="nladjs")
            prevs = cload([1, 2 * BPC], F32, prev_d, tag="prevs")
            t0ts = cload([128, 128], F32, t0t_d, tag="t0ts")
            pvecs = cload([1, 128], F32, pvec_d, tag="pvecs")
            avecs = cload([128, 1], F32, avec_d, tag="avecs")
            ident128 = cload([128, 128], BF16, ident_d, tag="ident128")
            oness = cload([1, 128], BF16, ones_d, tag="oness") if (nz_b1 or nz_b2 or nz_b3) else None
            b1s = cload([1, H], BF16, b1_d, tag="b1s") if nz_b1 else None
            b2s = cload([1, H2], BF16, b2_d, tag="b2s") if nz_b2 else None
            b3s = cload([1, 2], BF16, b3_d, tag="b3s") if nz_b3 else None
            g1s = cload([128, H], F32, g1_d, tag="g1s") if gb1 else None
            bt1s = cload([128, H], F32, bt1_d, tag="bt1s") if gb1 else None
            g2s = cload([128, H2], F32, g2_d, tag="g2s") if gb2 else None
            bt2s = cload([128, H2], F32, bt2_d, tag="bt2s") if gb2 else None

            # preload the one activation table that covers every function
            # this kernel uses (Gelu, Tanh, Identity) so the table-load pass
            # doesn't bounce through a Tanh-only table first
            nc.scalar.add_instruction(
                mybir.InstLoadActFuncSet(
                    name=nc.get_next_instruction_name(),
                    act_func_set_id=10,   # gelu_and_others
                    ins=[], outs=[],
                )
            )

            # ---- long-lived buffers
            rsqk = cpool.tile([128, QT], U32, tag="rsqk")
            nc.vector.memset(rsqk[:, :], 0x5F3759DF)
            ccat = hpool.tile([128, NT, 2], F32, tag="ccat")
            bases = hpool.tile([128, NT, 2], F32, tag="bases")
            finals = hpool.tile([128, NT, 2], F32, tag="finals")
            adjts = hpool.tile([128, NT, 2], F32, tag="adjts")
            zall = hpool.tile([128, NT, 2], F32, tag="zall")
            diffs = hpool.tile([128, NT], F32, tag="diffs")
            istd1 = hpool.tile([128, NT], F32, tag="istd1")
            nms1 = hpool.tile([128, NT], F32, tag="nms1")
            istd2 = hpool.tile([128, NT], F32, tag="istd2")
            nms2 = hpool.tile([128, NT], F32, tag="nms2")

            def sigma(st, istd_out, nms_out, n, eps, tag):
                """LN istd and -mean*istd for one quarter from bn_stats'
                per-tile (even,odd) half-stats.  Welford merge:
                M2 = M2e + M2o + (n/4)*(me-mo)^2 ; var = M2/n ;
                mean = (me+mo)/2.  istd = (var+eps)^-0.5 via magic-constant
                guess + one Newton step (rel err <= ~0.2%, far below the
                fp8 noise floor -- no ACT Sqrt table, one DVE op only)."""
                d = spool.tile([128, QT], F32, tag=tag + "d")
                nc.gpsimd.tensor_sub(d[:, :], st[:, :, 1], st[:, :, 4])
                nc.gpsimd.tensor_mul(d[:, :], d[:, :], d[:, :])
                nc.gpsimd.tensor_scalar_mul(d[:, :], d[:, :], n / 4.0)
                nc.gpsimd.tensor_add(d[:, :], d[:, :], st[:, :, 2])
                nc.gpsimd.tensor_add(d[:, :], d[:, :], st[:, :, 5])
                vp = spool.tile([128, QT], F32, tag=tag + "vp")
                nc.gpsimd.tensor_scalar(vp[:, :], d[:, :], 1.0 / n, eps, ALU.mult, ALU.add)
                y1 = spool.tile([128, QT], U32, tag=tag + "y1")
                nc.vector.tensor_scalar(
                    y1[:, :], vp[:, :].bitcast(U32), 1, None,
                    ALU.arith_shift_right,
                )
                y0u = spool.tile([128, QT], U32, tag=tag + "y0")
                nc.gpsimd.tensor_sub(y0u[:, :], rsqk[:, :], y1[:, :])
                y0 = y0u[:, :].bitcast(F32)
                t1 = spool.tile([128, QT], F32, tag=tag + "t1")
                nc.gpsimd.tensor_mul(t1[:, :], y0, y0)
                nc.gpsimd.tensor_mul(t1[:, :], t1[:, :], vp[:, :])
                nc.gpsimd.tensor_scalar(t1[:, :], t1[:, :], -0.5, 1.5, ALU.mult, ALU.add)
                nc.gpsimd.tensor_mul(istd_out, y0, t1[:, :])
                m = spool.tile([128, QT], F32, tag=tag + "m")
                nc.gpsimd.tensor_add(m[:, :], st[:, :, 1], st[:, :, 4])
                nc.gpsimd.tensor_mul(m[:, :], m[:, :], istd_out)
                nc.gpsimd.tensor_scalar_mul(nms_out, m[:, :], -0.5)

            def ema_batch(b):
                """Block-scan + final smoothing for batch b, all on PE.
                u[2j+ch] = sum_p A[p]*ccat[p, 16b+j, ch] is block j's
                end-of-block EMA contribution; the carry entering block j is
                S_{j-1} = u_{j-1} + 0.9^128 * S_{j-2}.  0.9^128 = 1.4e-6 and
                |S| <= 1, so the recursive term is below the fp8 noise floor
                by 3 orders of magnitude: the carry is just u shifted by one
                block (with `prev` entering block 0).
                finals = T0^T @ ccat + pvec (x) carry."""
                cslice = ccat[:, 16 * b : 16 * (b + 1), :]
                ema = pemapool.tile([128, 2, NTB, 2], F32, tag="ema")
                u_p = ema[:, 0]
                pfin = ema[:, 1]
                nc.tensor.matmul(
                    u_p[0:1, :, :], avecs[:, :], cslice,
                    start=True, stop=True,
                )
                u_sb = spool.tile([1, NTB, 2], F32, tag="u_sb")
                nc.vector.tensor_copy(u_sb[0:1, :, :], u_p[0:1, :, :])

                nc.tensor.matmul(
                    pfin[:, :, :], t0ts[:, :], cslice,
                    start=True, stop=False,
                )
                nc.tensor.matmul(
                    pfin[:, 1:NTB, :], pvecs[0:1, :], u_sb[0:1, 0 : NTB - 1, :],
                    start=False, stop=False,
                )
                nc.tensor.matmul(
                    pfin[:, 0:1, :], pvecs[0:1, :],
                    prevs[0:1, 2 * b : 2 * (b + 1)],
                    start=False, stop=True,
                )
                nc.vector.tensor_copy(finals[:, 16 * b : 16 * (b + 1), :], pfin[:, :, :])

            for q in range(NQ):
                q4 = q * QT
                xt = xtpool.tile([128, KC, TG], FP8, tag="xt")
                nc.sync.dma_start(xt[:, :, :], x_d[:, :, q * TG : (q + 1) * TG])

                # ---- layer 1 (fp8 DoubleRow) + stats
                ph1 = ph1pool.tile([128, QT, H], F32)
                st1 = spool.tile([128, QT, 6], F32, tag="bnst")
                for j in range(QT):
                    for kk in range(KC // 2):
                        nc.tensor.matmul(
                            ph1[:, j, :],
                            xt[:, 2 * kk : 2 * kk + 2, ts(j, 128)],
                            w1s[:, 2 * kk : 2 * kk + 2, :],
                            start=(kk == 0),
                            stop=(kk == KC // 2 - 1 and not nz_b1),
                            perf_mode=PM.DoubleRow,
                        )
                    if nz_b1:
                        nc.tensor.matmul(
                            ph1[:, j, :], oness[:, :], b1s[:, :],
                            start=False, stop=True,
                        )
                    nc.vector.bn_stats(st1[:, j, :], ph1[:, j, :])

                # ---- LN1 sigma for the quarter (gpsimd Welford merge of
                # bn_stats' even/odd halves; no ACT table, almost no DVE)
                sl = slice(q4, q4 + QT)
                sigma(st1, istd1[:, sl], nms1[:, sl], H, EPS1, "s1")

                # ---- LN1-apply + GELU (ACT from PSUM), transpose, layer 2
                ph2 = ph2pool.tile([128, QT, H2], F32)
                st2 = spool.tile([128, QT, 6], F32, tag="bnst2")
                for j in range(QT):
                    i = q4 + j
                    h1g = wpool.tile([128, H], BF16, tag="h1g")
                    if not gb1:
                        nc.scalar.activation(
                            h1g[:, :], ph1[:, j, :], AFT.Gelu,
                            bias=nms1[:, i : i + 1], scale=istd1[:, i : i + 1],
                        )
                    else:
                        tmp = spool.tile([128, H], F32, tag="lng1")
                        nc.scalar.activation(
                            tmp[:, :], ph1[:, j, :], AFT.Identity,
                            bias=nms1[:, i : i + 1], scale=istd1[:, i : i + 1],
                        )
                        nc.vector.tensor_mul(tmp[:, :], tmp[:, :], g1s[:, :])
                        nc.vector.tensor_add(tmp[:, :], tmp[:, :], bt1s[:, :])
                        nc.scalar.activation(h1g[:, :], tmp[:, :], AFT.Gelu)
                    h1gt = wpool.tile([128, 2, H2], BF16, tag="h1gt")
                    nc.sync.dma_start(h1gt[:, :, :], h1g[:, :], transpose=True)
                    for hh in range(2):
                        nc.tensor.matmul(
                            ph2[:, j, :], h1gt[:, hh, :], w2s[:, hh, :],
                            start=(hh == 0), stop=(hh == 1 and not nz_b2),
                        )
                    if nz_b2:
                        nc.tensor.matmul(
                            ph2[:, j, :], oness[:, :], b2s[:, :],
                            start=False, stop=True,
                        )
                    nc.vector.bn_stats(st2[:, j, :], ph2[:, j, :])

                # ---- LN2 sigma
                sigma(st2, istd2[:, sl], nms2[:, sl], H2, EPS, "s2")

                # ---- LN2-apply + GELU, transpose (PE), layer 3
                pl3t = pl3pool.tile([128, QT, 72], F32)
                pl3 = pl3t[:, :, 0:2]
                for j in range(QT):
                    i = q4 + j
                    h2g = wpool.tile([128, H2], BF16, tag="h2g")
                    if not gb2:
                        nc.scalar.activation(
                            h2g[:, :], ph2[:, j, :], AFT.Gelu,
                            bias=nms2[:, i : i + 1], scale=istd2[:, i : i + 1],
                        )
                    else:
                        tmp = spool.tile([128, H2], F32, tag="lng2")
                        nc.scalar.activation(
                            tmp[:, :], ph2[:, j, :], AFT.Identity,
                            bias=nms2[:, i : i + 1], scale=istd2[:, i : i + 1],
                        )
                        nc.vector.tensor_mul(tmp[:, :], tmp[:, :], g2s[:, :])
                        nc.vector.tensor_add(tmp[:, :], tmp[:, :], bt2s[:, :])
                        nc.scalar.activation(h2g[:, :], tmp[:, :], AFT.Gelu)
                    h2gtp = pl3t[:, j, 8:72].bitcast(BF16)
                    nc.tensor.matmul(
                        h2gtp, h2g[:, :], ident128[:, :],
                        is_transpose=True, start=True, stop=True,
                    )
                    h2gt = wpool.tile([128, H2], BF16, tag="h2gt")
                    nc.vector.tensor_copy(h2gt[:, :], h2gtp)
                    nc.tensor.matmul(
                        pl3[:, j, :], h2gt[:, :], w3s[:, :],
                        start=True, stop=not nz_b3,
                    )
                    if nz_b3:
                        nc.tensor.matmul(
                            pl3[:, j, :], oness[:, :], b3s[:, :],
                            start=False, stop=True,
                        )

                # ---- batched softmax tail for the quarter
                nc.scalar.activation(adjts[:, sl, :], pl3[:, :, :], AFT.Tanh)
                nc.gpsimd.tensor_sub(
                    diffs[:, sl], adjts[:, sl, 1], adjts[:, sl, 0]
                )
                nc.gpsimd.tensor_scalar(
                    zall[:, sl, 1], diffs[:, sl], 0.5 * sig_scale, None, ALU.mult
                )
                nc.gpsimd.tensor_add(zall[:, sl, 1], zall[:, sl, 1], ladjs[:, sl])
                nc.gpsimd.tensor_scalar(
                    zall[:, sl, 0], diffs[:, sl], -0.5 * sig_scale, None, ALU.mult
                )
                nc.gpsimd.tensor_add(zall[:, sl, 0], zall[:, sl, 0], nladjs[:, sl])
                th = spool.tile([128, QT, 2], F32, tag="th")
                nc.scalar.activation(th[:, :, :], zall[:, sl, :], AFT.Tanh)
                nc.gpsimd.tensor_scalar(
                    ccat[:, sl, :], th[:, :, :], 0.5, 0.5, ALU.mult, ALU.add
                )

                if (q + 1) % (NTB // QT) == 0:
                    ema_batch((q + 1) // (NTB // QT) - 1)

            # ---- base weights, batched over all tiles
            nc.gpsimd.tensor_scalar(
                bases[:, :, 0], labts[:, :], -0.5, 0.75, ALU.mult, ALU.add
            )
            nc.gpsimd.tensor_scalar(
                bases[:, :, 1], labts[:, :], 0.5, 0.25, ALU.mult, ALU.add
            )

            # ---- store outputs
            nc.sync.dma_start(fin_d[:, :], finals[:, :, :])
            nc.sync.dma_start(bas_d[:, :], bases[:, :, :])
            nc.scalar.dma_start(adw_d[:, :], ccat[:, :, :])

    nc.compile()
    return nc


_NC_CACHE = {}


def _get_nc(flags):
    key = tuple(sorted(flags.items()))
    if key not in _NC_CACHE:
        _NC_CACHE[key] = _build_nc(flags)
    return _NC_CACHE[key]


def _ema_constants():
    """Constant tensors for the matmul-based EMA block scan (fp32)."""
    s, o = SMOOTH, 1.0 - SMOOTH
    dt = np.arange(128)
    dk = np.arange(128)
    expo = dt[None, :] - dk[:, None]
    t0t = np.where(expo >= 0, o * np.power(s, np.clip(expo, 0, None)), 0.0)
    pvec = np.power(s, dt + 1.0)
    avec = o * np.power(s, 127.0 - dk)
    return (
        t0t.astype(np.float32),
        pvec.astype(np.float32).reshape(1, 128),
        avec.astype(np.float32).reshape(128, 1),
    )


def prepare(critical_labels, action_tokens, prev_weights,
            W1, b1, g1, bt1, W2, b2, g2, bt2, W3, b3, temperature):
    """Host-side marshalling. Returns (nc, in_maps, postprocess)."""
    labels = np.asarray(critical_labels)
    x = np.asarray(action_tokens, dtype=np.float32)
    prev = np.asarray(prev_weights, dtype=np.float32)
    W1 = np.asarray(W1, dtype=np.float32)
    W2 = np.asarray(W2, dtype=np.float32)
    W3 = np.asarray(W3, dtype=np.float32)
    b1 = np.asarray(b1, dtype=np.float32)
    b2 = np.asarray(b2, dtype=np.float32)
    b3 = np.asarray(b3, dtype=np.float32)
    g1 = np.asarray(g1, dtype=np.float32)
    bt1 = np.asarray(bt1, dtype=np.float32)
    g2 = np.asarray(g2, dtype=np.float32)
    bt2 = np.asarray(bt2, dtype=np.float32)
    temp = float(np.clip(np.asarray(temperature, dtype=np.float32), 0.1, None))
    inv_t = 1.0 / temp

    flags = {
        "nz_b1": bool(np.any(b1 != 0)),
        "nz_b2": bool(np.any(b2 != 0)),
        "nz_b3": bool(np.any(b3 != 0)),
        "gb1": bool(np.any(g1 != 1) or np.any(bt1 != 0)),
        "gb2": bool(np.any(g2 != 1) or np.any(bt2 != 0)),
        "sig_scale": float(ADJ * inv_t),
    }
    nc = _get_nc(flags)

    w1r = np.ascontiguousarray(
        (W1 * W1SCALE).astype(F8).reshape(KC, 128, H).transpose(1, 0, 2)
    )
    w2r = np.ascontiguousarray(
        W2.astype(BF).reshape(2, 128, H2).transpose(1, 0, 2)
    )
    w3r = np.ascontiguousarray(W3.astype(BF))
    t0t, pvec, avec = _ema_constants()
    shared = {
        "w1": w1r, "w2": w2r, "w3": w3r,
        "t0t": t0t, "pvec": pvec, "avec": avec,
        "ident": np.eye(128, dtype=BF),
    }
    if flags["nz_b1"] or flags["nz_b2"] or flags["nz_b3"]:
        shared["onesr"] = np.ones((1, 128), dtype=BF)
    if flags["nz_b1"]:
        shared["b1r"] = (b1 * W1SCALE).astype(BF).reshape(1, H)
    if flags["nz_b2"]:
        shared["b2r"] = b2.astype(BF).reshape(1, H2)
    if flags["nz_b3"]:
        shared["b3r"] = b3.astype(BF).reshape(1, 2)
    if flags["gb1"]:
        shared["g1f"] = np.broadcast_to(g1.reshape(1, H), (128, H)).copy()
        shared["bt1f"] = np.broadcast_to(bt1.reshape(1, H), (128, H)).copy()
    if flags["gb2"]:
        shared["g2f"] = np.broadcast_to(g2.reshape(1, H2), (128, H2)).copy()
        shared["bt2f"] = np.broadcast_to(bt2.reshape(1, H2), (128, H2)).copy()

    lab_f = labels.astype(np.float32).reshape(N_CORES, BPC * T)
    # fp8-quantize then transpose to [core, 128, KC, TOK]
    x8 = x.reshape(N_CORES, TOK, KC, 128).astype(F8)
    xt8 = np.ascontiguousarray(x8.transpose(0, 3, 2, 1))
    prev_r = prev.reshape(N_CORES, BPC * 2)

    in_maps = []
    for c in range(N_CORES):
        m = dict(shared)
        m["x"] = xt8[c]
        labt = np.ascontiguousarray(lab_f[c].reshape(NT, 128).T)
        m["labt"] = labt
        ladj = np.ascontiguousarray((labt - 0.5) * inv_t * 0.5)
        m["ladj"] = ladj
        m["nladj"] = np.ascontiguousarray(-ladj)
        m["prevr"] = prev_r[c : c + 1]
        in_maps.append(m)

    def postprocess(results):
        outs = []
        for name in ("fin", "bas", "adw"):
            per_core = []
            for c in range(N_CORES):
                a = results[c][name].reshape(128, NT, 2)
                per_core.append(
                    np.ascontiguousarray(a.transpose(1, 0, 2)).reshape(BPC, T, 2)
                )
            outs.append(np.concatenate(per_core, axis=0))
        return tuple(outs)   # (final, base, adjusted)

    return nc, in_maps, postprocess


def kernel(**inputs):
    nc, in_maps, postprocess = prepare(**inputs)
    res = run_bass_kernel_spmd(nc, in_maps, core_ids=list(range(N_CORES)))
    return postprocess(res.results)


# revision 10
# speedup vs baseline: 1.1545x; 1.0288x over previous
"""Trainium2 Bass kernel for nn_BinaryLabelSoftRouter.

Reference computation (B=16, T=2048, D=2048, H=256):
    base = lookup[labels]                                   (B,T,2)
    h = gelu(LN(x @ W1 + b1) * g1 + bt1)
    h = gelu(LN(h @ W2 + b2) * g2 + bt2)
    adj = tanh(h @ W3 + b3) * 0.1
    adjusted = softmax((base + adj) / clip(temp, 0.1))      (B,T,2)
    final = EMA scan over T (s_t = 0.9 s_{t-1} + 0.1 c_t)   (B,T,2)
    returns (final, base, adjusted)

Strategy: data-parallel over B across 8 NeuronCores (2 batches/core).
v2 design (vs the 113.7us baseline):
  - x is quantized to fp8e4m3 and pre-transposed on the host to
    [128, KC, TOK]; layer-1 runs fp8 DoubleRow matmuls (K=256 per
    instruction at 0.5 cycles/row) -> ~4x less PE time and ~2x less
    DMA-device time than bf16 + on-device XBAR transpose.
    W1 is scaled by 128 before the fp8 cast to lift it out of the fp8
    subnormal range; layernorm makes the result scale-invariant (b1 is
    scaled identically when nonzero).
  - h1/h2 pre-activations stay resident in PSUM; the scalar engine
    applies LN+GELU directly from PSUM (no DVE stash copies).
  - inverse LN sigma comes from one DVE tensor_scalar (v+eps)^-0.5
    (AluOpType.pow), so the ACT engine only ever needs the
    gelu_and_others table (Gelu+Tanh+Identity): one table load total.
  - the per-tile softmax tail is batched per 4-tile quarter: one Tanh
    over [128,8], strided DVE ops, one Tanh over [128,8].
  - the EMA block-scan carry is computed entirely with tiny matmuls
    (carry row u = A^T ccat, then an in-row convolution with powers of
    0.9^128) -- no DRAM round-trip.
"""

import sys

sys.path.insert(0, "/opt/trn_rl_repo")

import numpy as np
import ml_dtypes

import concourse.bass as bass
import concourse.mybir as mybir
from concourse import bacc
from concourse.bass import ts
from concourse.tile import TileContext
from concourse.bass_utils import run_bass_kernel_spmd

F32 = mybir.dt.float32
U32 = mybir.dt.uint32
BF16 = mybir.dt.bfloat16
FP8 = mybir.dt.float8e4
AFT = mybir.ActivationFunctionType
ALU = mybir.AluOpType
PM = mybir.MatmulPerfMode
BF = ml_dtypes.bfloat16
F8 = ml_dtypes.float8_e4m3

B, T, D, H = 16, 2048, 2048, 256
H2 = H // 2
ADJ = 0.1
SMOOTH = 0.9
EPS = 1e-5
N_CORES = 8
BPC = B // N_CORES           # batches per core
TOK = BPC * T                # tokens per core
NT = TOK // 128              # 128-token tiles per core (32)
NTB = T // 128               # tiles per batch (16)
KC = D // 128                # k-chunks of 128 for layer 1 (16)
QT = 4                       # tiles per quarter
NQ = NT // QT                # quarters (8)
TG = QT * 128                # tokens per x slab (512)
W1SCALE = 128.0              # fp8 range lift for W1 (LN removes it)
EPS1 = EPS * W1SCALE * W1SCALE   # ln eps in the scaled-h1 domain


def _build_nc(flags):
    nz_b1 = flags["nz_b1"]
    nz_b2 = flags["nz_b2"]
    nz_b3 = flags["nz_b3"]
    gb1 = flags["gb1"]
    gb2 = flags["gb2"]
    sig_scale = flags["sig_scale"]   # 0.1 / temp

    nc = bacc.Bacc("TRN2", target_bir_lowering=False)

    x_d = nc.dram_tensor("x", [128, KC, TOK], FP8, kind="ExternalInput")
    w1_d = nc.dram_tensor("w1", [128, KC, H], FP8, kind="ExternalInput")
    w2_d = nc.dram_tensor("w2", [128, 2, H2], BF16, kind="ExternalInput")
    w3_d = nc.dram_tensor("w3", [128, 2], BF16, kind="ExternalInput")
    labt_d = nc.dram_tensor("labt", [128, NT], F32, kind="ExternalInput")
    ladj_d = nc.dram_tensor("ladj", [128, NT], F32, kind="ExternalInput")
    nladj_d = nc.dram_tensor("nladj", [128, NT], F32, kind="ExternalInput")
    prev_d = nc.dram_tensor("prevr", [1, 2 * BPC], F32, kind="ExternalInput")
    t0t_d = nc.dram_tensor("t0t", [128, 128], F32, kind="ExternalInput")
    pvec_d = nc.dram_tensor("pvec", [1, 128], F32, kind="ExternalInput")
    avec_d = nc.dram_tensor("avec", [128, 1], F32, kind="ExternalInput")
    ident_d = nc.dram_tensor("ident", [128, 128], BF16, kind="ExternalInput")
    if nz_b1 or nz_b2 or nz_b3:
        ones_d = nc.dram_tensor("onesr", [1, 128], BF16, kind="ExternalInput")
    if nz_b1:
        b1_d = nc.dram_tensor("b1r", [1, H], BF16, kind="ExternalInput")
    if nz_b2:
        b2_d = nc.dram_tensor("b2r", [1, H2], BF16, kind="ExternalInput")
    if nz_b3:
        b3_d = nc.dram_tensor("b3r", [1, 2], BF16, kind="ExternalInput")
    if gb1:
        g1_d = nc.dram_tensor("g1f", [128, H], F32, kind="ExternalInput")
        bt1_d = nc.dram_tensor("bt1f", [128, H], F32, kind="ExternalInput")
    if gb2:
        g2_d = nc.dram_tensor("g2f", [128, H2], F32, kind="ExternalInput")
        bt2_d = nc.dram_tensor("bt2f", [128, H2], F32, kind="ExternalInput")

    fin_d = nc.dram_tensor("fin", [128, 2 * NT], F32, kind="ExternalOutput")
    bas_d = nc.dram_tensor("bas", [128, 2 * NT], F32, kind="ExternalOutput")
    adw_d = nc.dram_tensor("adw", [128, 2 * NT], F32, kind="ExternalOutput")

    with TileContext(nc) as tc:
        with (
            tc.tile_pool(name="consts", bufs=1) as cpool,
            tc.tile_pool(name="xt", bufs=8) as xtpool,
            tc.tile_pool(name="stash", bufs=1) as hpool,
            tc.tile_pool(name="work", bufs=6) as wpool,
            tc.tile_pool(name="small", bufs=8) as spool,
            tc.tile_pool(name="ph1", bufs=2, space="PSUM") as ph1pool,
            tc.tile_pool(name="ph2", bufs=2, space="PSUM") as ph2pool,
            tc.tile_pool(name="pl3", bufs=1, space="PSUM") as pl3pool,
            tc.tile_pool(name="pema", bufs=1, space="PSUM") as pemapool,
        ):
            # ---- constants into SBUF on the SWDGE queue: the Pool engine
            # is otherwise idle at kernel start, so its software descriptor
            # generation is free, and neither the sync queue (x slabs) nor
            # the Activation queue (GELUs) is delayed
            def cload(shape, dt, dram, tag):
                t = cpool.tile(shape, dt, tag=tag)
                sl = tuple(slice(None) for _ in shape)
                nc.gpsimd.dma_start(t[sl], dram[sl])
                return t

            w1s = cload([128, KC, H], FP8, w1_d, tag="w1s")
            w2s = cload([128, 2, H2], BF16, w2_d, tag="w2s")
            w3s = cload([128, 2], BF16, w3_d, tag="w3s")
            labts = cload([128, NT], F32, labt_d, tag="labts")
            ladjs = cload([128, NT], F32, ladj_d, tag="ladjs")
            nladjs = cload([128, NT], F32, nladj_d, tag="nladjs")
            prevs = cload([1, 2 * BPC], F32, prev_d, tag="prevs")
            t0ts = cload([128, 128], F32, t0t_d, tag="t0ts")
            pvecs = cload([1, 128], F32, pvec_d, tag="pvecs")
            avecs = cload([128, 1], F32, avec_d, tag="avecs")
            ident128 = cload([128, 128], BF16, ident_d, tag="ident128")
            oness = cload([1, 128], BF16, ones_d, tag="oness") if (nz_b1 or nz_b2 or nz_b3) else None
            b1s = cload([1, H], BF16, b1_d, tag="b1s") if nz_b1 else None
            b2s = cload([1, H2], BF16, b2_d, tag="b2s") if nz_b2 else None
            b3s = cload([1, 2], BF16, b3_d, tag="b3s") if nz_b3 else None
            g1s = cload([128, H], F32, g1_d, tag="g1s") if gb1 else None
            bt1s = cload([128, H], F32, bt1_d, tag="bt1s") if gb1 else None
            g2s = cload([128, H2], F32, g2_d, tag="g2s") if gb2 else None
            bt2s = cload([128, H2], F32, bt2_d, tag="bt2s") if gb2 else None

            # preload the one activation table that covers every function
            # this kernel uses (Gelu, Tanh, Identity) so the table-load pass
            # doesn't bounce through a Tanh-only table first
            nc.scalar.add_instruction(
                mybir.InstLoadActFuncSet(
                    name=nc.get_next_instruction_name(),
                    act_func_set_id=10,   # gelu_and_others
                    ins=[], outs=[],
                )
            )

            # ---- long-lived buffers
            rsqk = cpool.tile([128, QT], U32, tag="rsqk")
            nc.vector.memset(rsqk[:, :], 0x5F3759DF)
            ccat = hpool.tile([128, NT, 2], F32, tag="ccat")
            bases = hpool.tile([128, NT, 2], F32, tag="bases")
            finals = hpool.tile([128, NT, 2], F32, tag="finals")
            adjts = hpool.tile([128, NT, 2], F32, tag="adjts")
            zall = hpool.tile([128, NT, 2], F32, tag="zall")
            diffs = hpool.tile([128, NT], F32, tag="diffs")
            istd1 = hpool.tile([128, NT], F32, tag="istd1")
            nms1 = hpool.tile([128, NT], F32, tag="nms1")
            istd2 = hpool.tile([128, NT], F32, tag="istd2")
            nms2 = hpool.tile([128, NT], F32, tag="nms2")

            def sigma(st, istd_out, nms_out, n, eps, tag):
                """LN istd and -mean*istd for one quarter from bn_stats'
                per-tile (even,odd) half-stats.  Welford merge:
                M2 = M2e + M2o + (n/4)*(me-mo)^2 ; var = M2/n ;
                mean = (me+mo)/2.  istd = (var+eps)^-0.5 via magic-constant
                guess + one Newton step (rel err <= ~0.2%, far below the
                fp8 noise floor -- no ACT Sqrt table, one DVE op only)."""
                d = spool.tile([128, QT], F32, tag=tag + "d")
                nc.gpsimd.tensor_sub(d[:, :], st[:, :, 1], st[:, :, 4])
                nc.gpsimd.tensor_mul(d[:, :], d[:, :], d[:, :])
                nc.gpsimd.tensor_scalar_mul(d[:, :], d[:, :], n / 4.0)
                nc.gpsimd.tensor_add(d[:, :], d[:, :], st[:, :, 2])
                nc.gpsimd.tensor_add(d[:, :], d[:, :], st[:, :, 5])
                vp = spool.tile([128, QT], F32, tag=tag + "vp")
                nc.gpsimd.tensor_scalar(vp[:, :], d[:, :], 1.0 / n, eps, ALU.mult, ALU.add)
                y1 = spool.tile([128, QT], U32, tag=tag + "y1")
                nc.vector.tensor_scalar(
                    y1[:, :], vp[:, :].bitcast(U32), 1, None,
                    ALU.arith_shift_right,
                )
                y0u = spool.tile([128, QT], U32, tag=tag + "y0")
                nc.gpsimd.tensor_sub(y0u[:, :], rsqk[:, :], y1[:, :])
                y0 = y0u[:, :].bitcast(F32)
                t1 = spool.tile([128, QT], F32, tag=tag + "t1")
                nc.gpsimd.tensor_mul(t1[:, :], y0, y0)
                nc.gpsimd.tensor_mul(t1[:, :], t1[:, :], vp[:, :])
                nc.gpsimd.tensor_scalar(t1[:, :], t1[:, :], -0.5, 1.5, ALU.mult, ALU.add)
                nc.gpsimd.tensor_mul(istd_out, y0, t1[:, :])
                m = spool.tile([128, QT], F32, tag=tag + "m")
                nc.gpsimd.tensor_add(m[:, :], st[:, :, 1], st[:, :, 4])
                nc.gpsimd.tensor_mul(m[:, :], m[:, :], istd_out)
                nc.gpsimd.tensor_scalar_mul(nms_out, m[:, :], -0.5)

            def ema_batch(b):
                """Block-scan + final smoothing for batch b, all on PE.
                u[2j+ch] = sum_p A[p]*ccat[p, 16b+j, ch] is block j's
                end-of-block EMA contribution; the carry entering block j is
                S_{j-1} = u_{j-1} + 0.9^128 * S_{j-2}.  0.9^128 = 1.4e-6 and
                |S| <= 1, so the recursive term is below the fp8 noise floor
                by 3 orders of magnitude: the carry is just u shifted by one
                block (with `prev` entering block 0).
                finals = T0^T @ ccat + pvec (x) carry."""
                cslice = ccat[:, 16 * b : 16 * (b + 1), :]
                ema = pemapool.tile([128, 2, NTB, 2], F32, tag="ema")
                u_p = ema[:, 0]
                pfin = ema[:, 1]
                nc.tensor.matmul(
                    u_p[0:1, :, :], avecs[:, :], cslice,
                    start=True, stop=True,
                )
                u_sb = spool.tile([1, NTB, 2], F32, tag="u_sb")
                nc.vector.tensor_copy(u_sb[0:1, :, :], u_p[0:1, :, :])

                nc.tensor.matmul(
                    pfin[:, :, :], t0ts[:, :], cslice,
                    start=True, stop=False,
                )
                nc.tensor.matmul(
                    pfin[:, 1:NTB, :], pvecs[0:1, :], u_sb[0:1, 0 : NTB - 1, :],
                    start=False, stop=False,
                )
                nc.tensor.matmul(
                    pfin[:, 0:1, :], pvecs[0:1, :],
                    prevs[0:1, 2 * b : 2 * (b + 1)],
                    start=False, stop=True,
                )
                nc.vector.tensor_copy(finals[:, 16 * b : 16 * (b + 1), :], pfin[:, :, :])

            # interleave the two batches' independent pipelines so each
            # engine always has work from the other stream to fill
            # dependency bubbles (q0,q4,q1,q5,... instead of q0..q7)
            halfq = NQ // 2
            qorder = [q for pair in zip(range(halfq), range(halfq, NQ)) for q in pair]
            for q in qorder:
                q4 = q * QT
                xt = xtpool.tile([128, KC, TG], FP8, tag="xt")
                nc.sync.dma_start(xt[:, :, :], x_d[:, :, q * TG : (q + 1) * TG])

                # ---- layer 1 (fp8 DoubleRow) + stats
                ph1 = ph1pool.tile([128, QT, H], F32)
                st1 = spool.tile([128, QT, 6], F32, tag="bnst")
                for j in range(QT):
                    for kk in range(KC // 2):
                        nc.tensor.matmul(
                            ph1[:, j, :],
                            xt[:, 2 * kk : 2 * kk + 2, ts(j, 128)],
                            w1s[:, 2 * kk : 2 * kk + 2, :],
                            start=(kk == 0),
                            stop=(kk == KC // 2 - 1 and not nz_b1),
                            perf_mode=PM.DoubleRow,
                        )
                    if nz_b1:
                        nc.tensor.matmul(
                            ph1[:, j, :], oness[:, :], b1s[:, :],
                            start=False, stop=True,
                        )
                    nc.vector.bn_stats(st1[:, j, :], ph1[:, j, :])

                # ---- LN1 sigma for the quarter (gpsimd Welford merge of
                # bn_stats' even/odd halves; no ACT table, almost no DVE)
                sl = slice(q4, q4 + QT)
                sigma(st1, istd1[:, sl], nms1[:, sl], H, EPS1, "s1")

                # ---- LN1-apply + GELU (ACT from PSUM), transpose, layer 2
                ph2 = ph2pool.tile([128, QT, H2], F32)
                st2 = spool.tile([128, QT, 6], F32, tag="bnst2")
                for j in range(QT):
                    i = q4 + j
                    h1g = wpool.tile([128, H], BF16, tag="h1g")
                    if not gb1:
                        nc.scalar.activation(
                            h1g[:, :], ph1[:, j, :], AFT.Gelu,
                            bias=nms1[:, i : i + 1], scale=istd1[:, i : i + 1],
                        )
                    else:
                        tmp = spool.tile([128, H], F32, tag="lng1")
                        nc.scalar.activation(
                            tmp[:, :], ph1[:, j, :], AFT.Identity,
                            bias=nms1[:, i : i + 1], scale=istd1[:, i : i + 1],
                        )
                        nc.vector.tensor_mul(tmp[:, :], tmp[:, :], g1s[:, :])
                        nc.vector.tensor_add(tmp[:, :], tmp[:, :], bt1s[:, :])
                        nc.scalar.activation(h1g[:, :], tmp[:, :], AFT.Gelu)
                    h1gt = wpool.tile([128, 2, H2], BF16, tag="h1gt")
                    nc.sync.dma_start(h1gt[:, :, :], h1g[:, :], transpose=True)
                    for hh in range(2):
                        nc.tensor.matmul(
                            ph2[:, j, :], h1gt[:, hh, :], w2s[:, hh, :],
                            start=(hh == 0), stop=(hh == 1 and not nz_b2),
                        )
                    if nz_b2:
                        nc.tensor.matmul(
                            ph2[:, j, :], oness[:, :], b2s[:, :],
                            start=False, stop=True,
                        )
                    nc.vector.bn_stats(st2[:, j, :], ph2[:, j, :])

                # ---- LN2 sigma
                sigma(st2, istd2[:, sl], nms2[:, sl], H2, EPS, "s2")

                # ---- LN2-apply + GELU, transpose (PE), layer 3
                pl3t = pl3pool.tile([128, QT, 72], F32)
                pl3 = pl3t[:, :, 0:2]
                for j in range(QT):
                    i = q4 + j
                    h2g = wpool.tile([128, H2], BF16, tag="h2g")
                    if not gb2:
                        nc.scalar.activation(
                            h2g[:, :], ph2[:, j, :], AFT.Gelu,
                            bias=nms2[:, i : i + 1], scale=istd2[:, i : i + 1],
                        )
                    else:
                        tmp = spool.tile([128, H2], F32, tag="lng2")
                        nc.scalar.activation(
                            tmp[:, :], ph2[:, j, :], AFT.Identity,
                            bias=nms2[:, i : i + 1], scale=istd2[:, i : i + 1],
                        )
                        nc.vector.tensor_mul(tmp[:, :], tmp[:, :], g2s[:, :])
                        nc.vector.tensor_add(tmp[:, :], tmp[:, :], bt2s[:, :])
                        nc.scalar.activation(h2g[:, :], tmp[:, :], AFT.Gelu)
                    h2gtp = pl3t[:, j, 8:72].bitcast(BF16)
                    nc.tensor.matmul(
                        h2gtp, h2g[:, :], ident128[:, :],
                        is_transpose=True, start=True, stop=True,
                    )
                    h2gt = wpool.tile([128, H2], BF16, tag="h2gt")
                    nc.vector.tensor_copy(h2gt[:, :], h2gtp)
                    nc.tensor.matmul(
                        pl3[:, j, :], h2gt[:, :], w3s[:, :],
                        start=True, stop=not nz_b3,
                    )
                    if nz_b3:
                        nc.tensor.matmul(
                            pl3[:, j, :], oness[:, :], b3s[:, :],
                            start=False, stop=True,
                        )

                # ---- batched softmax tail for the quarter
                nc.scalar.activation(adjts[:, sl, :], pl3[:, :, :], AFT.Tanh)
                nc.gpsimd.tensor_sub(
                    diffs[:, sl], adjts[:, sl, 1], adjts[:, sl, 0]
                )
                nc.gpsimd.tensor_scalar(
                    zall[:, sl, 1], diffs[:, sl], 0.5 * sig_scale, None, ALU.mult
                )
                nc.gpsimd.tensor_add(zall[:, sl, 1], zall[:, sl, 1], ladjs[:, sl])
                nc.gpsimd.tensor_scalar(
                    zall[:, sl, 0], diffs[:, sl], -0.5 * sig_scale, None, ALU.mult
                )
                nc.gpsimd.tensor_add(zall[:, sl, 0], zall[:, sl, 0], nladjs[:, sl])
                th = spool.tile([128, QT, 2], F32, tag="th")
                nc.scalar.activation(th[:, :, :], zall[:, sl, :], AFT.Tanh)
                nc.gpsimd.tensor_scalar(
                    ccat[:, sl, :], th[:, :, :], 0.5, 0.5, ALU.mult, ALU.add
                )

                if q == halfq - 1:
                    ema_batch(0)
                elif q == NQ - 1:
                    ema_batch(1)

            # ---- base weights, batched over all tiles
            nc.gpsimd.tensor_scalar(
                bases[:, :, 0], labts[:, :], -0.5, 0.75, ALU.mult, ALU.add
            )
            nc.gpsimd.tensor_scalar(
                bases[:, :, 1], labts[:, :], 0.5, 0.25, ALU.mult, ALU.add
            )

            # ---- store outputs
            nc.sync.dma_start(fin_d[:, :], finals[:, :, :])
            nc.sync.dma_start(bas_d[:, :], bases[:, :, :])
            nc.scalar.dma_start(adw_d[:, :], ccat[:, :, :])

    nc.compile()
    return nc


_NC_CACHE = {}


def _get_nc(flags):
    key = tuple(sorted(flags.items()))
    if key not in _NC_CACHE:
        _NC_CACHE[key] = _build_nc(flags)
    return _NC_CACHE[key]


def _ema_constants():
    """Constant tensors for the matmul-based EMA block scan (fp32)."""
    s, o = SMOOTH, 1.0 - SMOOTH
    dt = np.arange(128)
    dk = np.arange(128)
    expo = dt[None, :] - dk[:, None]
    t0t = np.where(expo >= 0, o * np.power(s, np.clip(expo, 0, None)), 0.0)
    pvec = np.power(s, dt + 1.0)
    avec = o * np.power(s, 127.0 - dk)
    return (
        t0t.astype(np.float32),
        pvec.astype(np.float32).reshape(1, 128),
        avec.astype(np.float32).reshape(128, 1),
    )


def prepare(critical_labels, action_tokens, prev_weights,
            W1, b1, g1, bt1, W2, b2, g2, bt2, W3, b3, temperature):
    """Host-side marshalling. Returns (nc, in_maps, postprocess)."""
    labels = np.asarray(critical_labels)
    x = np.asarray(action_tokens, dtype=np.float32)
    prev = np.asarray(prev_weights, dtype=np.float32)
    W1 = np.asarray(W1, dtype=np.float32)
    W2 = np.asarray(W2, dtype=np.float32)
    W3 = np.asarray(W3, dtype=np.float32)
    b1 = np.asarray(b1, dtype=np.float32)
    b2 = np.asarray(b2, dtype=np.float32)
    b3 = np.asarray(b3, dtype=np.float32)
    g1 = np.asarray(g1, dtype=np.float32)
    bt1 = np.asarray(bt1, dtype=np.float32)
    g2 = np.asarray(g2, dtype=np.float32)
    bt2 = np.asarray(bt2, dtype=np.float32)
    temp = float(np.clip(np.asarray(temperature, dtype=np.float32), 0.1, None))
    inv_t = 1.0 / temp

    flags = {
        "nz_b1": bool(np.any(b1 != 0)),
        "nz_b2": bool(np.any(b2 != 0)),
        "nz_b3": bool(np.any(b3 != 0)),
        "gb1": bool(np.any(g1 != 1) or np.any(bt1 != 0)),
        "gb2": bool(np.any(g2 != 1) or np.any(bt2 != 0)),
        "sig_scale": float(ADJ * inv_t),
    }
    nc = _get_nc(flags)

    w1r = np.ascontiguousarray(
        (W1 * W1SCALE).astype(F8).reshape(KC, 128, H).transpose(1, 0, 2)
    )
    w2r = np.ascontiguousarray(
        W2.astype(BF).reshape(2, 128, H2).transpose(1, 0, 2)
    )
    w3r = np.ascontiguousarray(W3.astype(BF))
    t0t, pvec, avec = _ema_constants()
    shared = {
        "w1": w1r, "w2": w2r, "w3": w3r,
        "t0t": t0t, "pvec": pvec, "avec": avec,
        "ident": np.eye(128, dtype=BF),
    }
    if flags["nz_b1"] or flags["nz_b2"] or flags["nz_b3"]:
        shared["onesr"] = np.ones((1, 128), dtype=BF)
    if flags["nz_b1"]:
        shared["b1r"] = (b1 * W1SCALE).astype(BF).reshape(1, H)
    if flags["nz_b2"]:
        shared["b2r"] = b2.astype(BF).reshape(1, H2)
    if flags["nz_b3"]:
        shared["b3r"] = b3.astype(BF).reshape(1, 2)
    if flags["gb1"]:
        shared["g1f"] = np.broadcast_to(g1.reshape(1, H), (128, H)).copy()
        shared["bt1f"] = np.broadcast_to(bt1.reshape(1, H), (128, H)).copy()
    if flags["gb2"]:
        shared["g2f"] = np.broadcast_to(g2.reshape(1, H2), (128, H2)).copy()
        shared["bt2f"] = np.broadcast_to(bt2.reshape(1, H2), (128, H2)).copy()

    lab_f = labels.astype(np.float32).reshape(N_CORES, BPC * T)
    # fp8-quantize then transpose to [core, 128, KC, TOK]
    x8 = x.reshape(N_CORES, TOK, KC, 128).astype(F8)
    xt8 = np.ascontiguousarray(x8.transpose(0, 3, 2, 1))
    prev_r = prev.reshape(N_CORES, BPC * 2)

    in_maps = []
    for c in range(N_CORES):
        m = dict(shared)
        m["x"] = xt8[c]
        labt = np.ascontiguousarray(lab_f[c].reshape(NT, 128).T)
        m["labt"] = labt
        ladj = np.ascontiguousarray((labt - 0.5) * inv_t * 0.5)
        m["ladj"] = ladj
        m["nladj"] = np.ascontiguousarray(-ladj)
        m["prevr"] = prev_r[c : c + 1]
        in_maps.append(m)

    def postprocess(results):
        outs = []
        for name in ("fin", "bas", "adw"):
            per_core = []
            for c in range(N_CORES):
                a = results[c][name].reshape(128, NT, 2)
                per_core.append(
                    np.ascontiguousarray(a.transpose(1, 0, 2)).reshape(BPC, T, 2)
                )
            outs.append(np.concatenate(per_core, axis=0))
        return tuple(outs)   # (final, base, adjusted)

    return nc, in_maps, postprocess


def kernel(**inputs):
    nc, in_maps, postprocess = prepare(**inputs)
    res = run_bass_kernel_spmd(nc, in_maps, core_ids=list(range(N_CORES)))
    return postprocess(res.results)
